# revision 2
# baseline (speedup 1.0000x reference)
"""DepthRelationEmbedding Trainium2 kernel.

Math: out[h, n, hw] = relu( sum_d pos[n,hw,d] * W[d,h] + b[h] ) where pos is the
interleaved sin/cos embedding of delta[n,hw] = ln((relu(pd[n])+eps)/(dm[hw]+eps)).

Key identity: the embedding angle separates: angle_k(n,hw) = A_k(n) - C_k(hw)
with A_k = s_k*ln(relu(pd)+eps), C_k = s_k*ln(dm+eps). Using angle addition the
(N, HW, 256) intermediate never exists:
  out[n,hw,h] = sum_k U[k,(n,h)]*cosC[k,hw] + V[k,(n,h)]*sinC[k,hw]
  U = sinA*We + cosA*Wo,  V = sinA*Wo - cosA*We   (We = W[0::2], Wo = W[1::2])
which is one (256 x M) @ (256 x HW) matmul per core.

Trig arguments reach +-1600 rad; ACT Sin is only valid on [-pi, pi], so angles
are computed in "turns" (tau = angle/2pi) via a K=6 bf16-split outer-product
matmul (exact to ~2^-24), range-reduced with f = tau - rint(tau) (the DVE
fp32->int32 copy rounds to nearest on HW), and evaluated as
  sin(2pi tau) = Sin(2pi*f),  cos(2pi tau) = Sin(pi/2 - 2pi*|f|).

Sharding: SN x SH = 4 x 2 cores over (N, HW). Each core computes a full
[M=n_pad*8, hw_per] output block; host reassembles.
"""

import numpy as np

import sys

for p in ("/opt/trn_rl_repo", "/root/.axon_site/_ro/trn_rl_repo"):
    if p not in sys.path:
        sys.path.insert(0, p)

import ml_dtypes
from contextlib import ExitStack

from concourse import bacc, mybir, tile
from concourse.bass_utils import run_bass_kernel_spmd

F32 = mybir.dt.float32
F32R = mybir.dt.float32r
BF16 = mybir.dt.bfloat16
I32 = mybir.dt.int32
A = mybir.AluOpType
AF = mybir.ActivationFunctionType

# ---- problem constants (hardcoded; kernel.py must be self-contained) ----
N_TOT, H_DM, W_DM = 300, 24, 80
HW_TOT = H_DM * W_DM  # 1920
HEADS = 8
ED = 256  # embed dim
K = ED // 2  # 128 frequencies
EPS = 1e-5
SCALE = 100.0
TEMPERATURE = 10000.0
TWO_PI = 2.0 * np.pi

# ---- sharding config ----
SN, SH = 4, 2  # cores = SN * SH = 8
n_per = -(-N_TOT // SN)  # 75
n_pad = n_per + (n_per % 2)  # 76 (even, for clean [dm_rows, pd_cols] packing)
hw_per = HW_TOT // SH  # 960
dm_rows = 128 // SH  # 64
pd_cols = -(-n_pad // dm_rows)  # 2
M = n_pad * HEADS  # 608
CH = 480  # hw chunk width (psum-bank sized)
n_chunks = hw_per // CH
LPW = 15 + pd_cols  # logpack width

_m_tiles = []
_ms = 0
while _ms < M:
    _m_tiles.append((_ms, min(128, M - _ms)))
    _ms += 128


def _sigma_splits():
    k = np.arange(K)
    dim_t = (TEMPERATURE ** (k.astype(np.float32) * 2.0 / ED)).astype(np.float32)
    sigma = (SCALE / dim_t.astype(np.float64)) / TWO_PI
    s0 = sigma.astype(ml_dtypes.bfloat16)
    r1 = sigma - s0.astype(np.float64)
    s1 = r1.astype(ml_dtypes.bfloat16)
    r2 = r1 - s1.astype(np.float64)
    s2 = r2.astype(ml_dtypes.bfloat16)
    # rhs rows are [c0,c0,c0,c1,c1,c2]; pair with sigma rows below
    return np.stack([s0, s1, s2, s0, s1, s0]).astype(ml_dtypes.bfloat16)  # [6,128]


def _build_program():
    nc = bacc.Bacc("TRN2", target_bir_lowering=False, debug=False)

    lp_d = nc.dram_tensor("logpack", [dm_rows, LPW], F32, kind="ExternalInput")
    we_d = nc.dram_tensor("we", [K, HEADS], F32, kind="ExternalInput")
    wo_d = nc.dram_tensor("wo", [K, HEADS], F32, kind="ExternalInput")
    bias_d = nc.dram_tensor("bias_rep", [128, 1], F32, kind="ExternalInput")
    out_d = nc.dram_tensor("out", [M, hw_per], F32, kind="ExternalOutput")
    sig_d = nc.inline_tensor(np.ascontiguousarray(_sigma_splits()), name="sigma6")

    with tile.TileContext(nc) as tc, ExitStack() as ctx:
        sb = ctx.enter_context(tc.tile_pool(name="sb", bufs=1))
        sb2 = ctx.enter_context(tc.tile_pool(name="sb2", bufs=2))
        ps = ctx.enter_context(tc.tile_pool(name="ps", bufs=1, space="PSUM"))
        ps2 = ctx.enter_context(tc.tile_pool(name="ps2", bufs=2, space="PSUM"))
        pso = ctx.enter_context(tc.tile_pool(name="pso", bufs=4, space="PSUM"))

        # ---- constants ----
        def const_tile(val, tag):
            t = sb.tile((128, 1), F32, tag=tag)
            nc.vector.memset(t[:], val)
            return t

        twopi_c = const_tile(TWO_PI, "c_2pi")
        negtwopi_c = const_tile(-TWO_PI, "c_n2pi")
        halfpi_c = const_tile(np.pi / 2, "c_hpi")

        lhs_s = sb.tile((6, K), BF16, tag="lhs_s")
        nc.sync.dma_start(lhs_s[:], sig_d[:])
        we_t = sb.tile((K, HEADS), F32, tag="we")
        nc.sync.dma_start(we_t[:], we_d[:])
        wo_t = sb.tile((K, HEADS), F32, tag="wo")
        nc.sync.dma_start(wo_t[:], wo_d[:])
        bias_t = sb.tile((128, 1), F32, tag="bias")
        nc.sync.dma_start(bias_t[:], bias_d[:])

        # ---- logs: lp = [dm | pd] packed; relu(pd), +eps, ln ----
        lp = sb.tile((dm_rows, LPW), F32, tag="lp")
        nc.sync.dma_start(lp[:], lp_d[:])
        nc.vector.tensor_scalar(lp[:, 15:], lp[:, 15:], 0.0, None, A.max)
        nc.vector.tensor_scalar(lp[:], lp[:], EPS, None, A.add)
        lnv = sb.tile((dm_rows, LPW), F32, tag="lnv")
        nc.scalar.activation(lnv[:], lp[:], AF.Ln)

        # ---- 3-way bf16 split of the logs ----
        b0 = sb.tile((dm_rows, LPW), BF16, tag="b0")
        nc.vector.tensor_copy(b0[:], lnv[:])
        r1 = sb.tile((dm_rows, LPW), F32, tag="r1")
        nc.vector.tensor_tensor(r1[:], lnv[:], b0[:], A.subtract)
        b1 = sb.tile((dm_rows, LPW), BF16, tag="b1")
        nc.vector.tensor_copy(b1[:], r1[:])
        r2 = sb.tile((dm_rows, LPW), F32, tag="r2")
        nc.vector.tensor_tensor(r2[:], r1[:], b1[:], A.subtract)
        b2 = sb.tile((dm_rows, LPW), BF16, tag="b2")
        nc.vector.tensor_copy(b2[:], r2[:])

        # ---- assemble split-product rhs rows [c0,c0,c0,c1,c1,c2] ----
        rhs_c = sb.tile((6, hw_per), BF16, tag="rhs_c")
        rhs_a = sb.tile((6, n_pad), BF16, tag="rhs_a")
        p_a = n_pad // pd_cols
        for bt, lo, hi in ((b0, 0, 3), (b1, 3, 5), (b2, 5, 6)):
            for r in range(lo, hi):
                nc.sync.dma_start(
                    rhs_c[r : r + 1, :].rearrange("r (p j) -> r p j", j=15),
                    bt[:, 0:15],
                )
                nc.sync.dma_start(
                    rhs_a[r : r + 1, :].rearrange("r (p j) -> r p j", j=pd_cols),
                    bt[0:p_a, 15:LPW],
                )

        # ---- helper: tau psum -> (sin, cos) via range reduction ----
        def reduce_and_trig(ps_t, width, sin_ap, cos_ap, tag):
            q = sb2.tile((K, width), I32, tag=f"q{tag}")
            nc.vector.tensor_copy(q[:], ps_t[:])  # rint on HW
            f = sb2.tile((K, width), F32, tag=f"f{tag}")
            nc.vector.tensor_tensor(f[:], ps_t[:], q[:], A.subtract)
            u = sb2.tile((K, width), F32, tag=f"u{tag}")
            nc.vector.tensor_scalar(
                u[:].bitcast(I32), f[:].bitcast(I32), 0x7FFFFFFF, None, A.bitwise_and
            )
            nc.scalar.activation(sin_ap, f[:], AF.Sin, scale=twopi_c[:])
            nc.scalar.activation(
                cos_ap, u[:], AF.Sin, bias=halfpi_c[:], scale=negtwopi_c[:]
            )

        # ---- A-side grid ----
        ps_a = ps.tile((K, n_pad), F32, tag="psa")
        nc.tensor.matmul(ps_a[:], lhs_s[:], rhs_a[:], start=True, stop=True)
        sinA = sb.tile((K, n_pad), F32, tag="sinA")
        cosA = sb.tile((K, n_pad), F32, tag="cosA")
        reduce_and_trig(ps_a, n_pad, sinA[:], cosA[:], "a")

        # ---- T build: U = sinA*We + cosA*Wo, V = sinA*Wo - cosA*We ----
        U = sb.tile((K, M), F32R, tag="U")
        V = sb.tile((K, M), F32R, tag="V")
        tmp1 = sb.tile((K, M), F32, tag="tmp1")
        tmp2 = sb.tile((K, M), F32, tag="tmp2")

        def bcast_n(t):  # [K, n_pad] -> [K, n_pad, HEADS]
            return t[:].unsqueeze(2).to_broadcast((K, n_pad, HEADS))

        def bcast_h(t):  # [K, HEADS] -> [K, n_pad, HEADS]
            return t[:].unsqueeze(1).to_broadcast((K, n_pad, HEADS))

        def r3(t):  # [K, M] viewed as [K, n_pad, HEADS]
            return t[:].rearrange("p (n h) -> p n h", h=HEADS)

        nc.vector.tensor_tensor(r3(tmp1), bcast_n(sinA), bcast_h(we_t), A.mult)
        nc.vector.tensor_tensor(r3(tmp2), bcast_n(cosA), bcast_h(wo_t), A.mult)
        nc.vector.tensor_tensor(U[:], tmp1[:], tmp2[:], A.add)
        nc.vector.tensor_tensor(r3(tmp1), bcast_n(sinA), bcast_h(wo_t), A.mult)
        nc.vector.tensor_tensor(r3(tmp2), bcast_n(cosA), bcast_h(we_t), A.mult)
        nc.vector.tensor_tensor(V[:], tmp1[:], tmp2[:], A.subtract)

        # ---- C-side grid (chunked) + main matmul + relu + store ----
        cs_sin = sb.tile((K, hw_per), F32R, tag="cs_sin")
        cs_cos = sb.tile((K, hw_per), F32R, tag="cs_cos")
        for ci in range(n_chunks):
            sl = slice(ci * CH, (ci + 1) * CH)
            ps_c = ps2.tile((K, CH), F32, tag="psc")
            nc.tensor.matmul(ps_c[:], lhs_s[:], rhs_c[:, sl], start=True, stop=True)
            reduce_and_trig(ps_c, CH, cs_sin[:, sl], cs_cos[:, sl], "c")

            for ms, mr in _m_tiles:
                ps_o = pso.tile((128, CH), F32, tag="pso")
                nc.tensor.matmul(
                    ps_o[:mr, :], U[:, ms : ms + mr], cs_cos[:, sl],
                    start=True, stop=False,
                )
                nc.tensor.matmul(
                    ps_o[:mr, :], V[:, ms : ms + mr], cs_sin[:, sl],
                    start=False, stop=True,
                )
                ob = sb2.tile((128, CH), F32, tag="ob")
                nc.vector.tensor_scalar(
                    ob[:mr, :], ps_o[:mr, :], bias_t[0:mr], 0.0, A.add, A.max
                )
                nc.sync.dma_start(out_d[ms : ms + mr, sl], ob[:mr, :])

    nc.finalize()
    return nc


_NC = None


def _get_nc():
    global _NC
    if _NC is None:
        _NC = _build_program()
    return _NC


def _make_in_maps(predict_depth, depth_map, W, b):
    pd = np.asarray(predict_depth, np.float32).reshape(N_TOT)
    dm = np.asarray(depth_map, np.float32).reshape(128, 15)
    W = np.asarray(W, np.float32)
    b = np.asarray(b, np.float32)
    we = np.ascontiguousarray(W[0::2, :])
    wo = np.ascontiguousarray(W[1::2, :])
    bias_rep = np.ascontiguousarray(np.tile(b, 16)[:, None])

    in_maps = []
    for c in range(SN * SH):
        ni, hi = c // SH, c % SH
        pd_sl = pd[ni * n_per : ni * n_per + n_per]
        pd_pack = np.zeros(dm_rows * pd_cols, np.float32)
        pd_pack[: pd_sl.size] = pd_sl
        logpack = np.concatenate(
            [dm[hi * dm_rows : (hi + 1) * dm_rows], pd_pack.reshape(dm_rows, pd_cols)],
            axis=1,
        )
        in_maps.append(
            {
                "logpack": np.ascontiguousarray(logpack),
                "we": we,
                "wo": wo,
                "bias_rep": bias_rep,
            }
        )
    return in_maps


def _run(inputs, trace=False):
    nc = _get_nc()
    in_maps = _make_in_maps(**inputs)
    res = run_bass_kernel_spmd(
        nc, in_maps, core_ids=list(range(SN * SH)), trace=trace
    )
    out = np.empty((HEADS, N_TOT, HW_TOT), np.float32)
    for c in range(SN * SH):
        ni, hi = c // SH, c % SH
        blk = res.results[c]["out"].reshape(n_pad, HEADS, hw_per).transpose(1, 0, 2)
        n0 = ni * n_per
        n_cnt = min(n_per, N_TOT - n0)
        out[:, n0 : n0 + n_cnt, hi * hw_per : (hi + 1) * hw_per] = blk[:, :n_cnt, :]
    return out, res


def kernel(predict_depth, depth_map, W, b):
    out, _ = _run(
        {"predict_depth": predict_depth, "depth_map": depth_map, "W": W, "b": b}
    )
    return out


# revision 7
# speedup vs baseline: 1.0868x; 1.0868x over previous
"""DepthRelationEmbedding Trainium2 kernel.

Math: out[h, n, hw] = relu( sum_d pos[n,hw,d] * W[d,h] + b[h] ) where pos is the
interleaved sin/cos embedding of delta[n,hw] = ln((relu(pd[n])+eps)/(dm[hw]+eps)).

Key identity: the embedding angle separates: angle_k(n,hw) = A_k(n) - C_k(hw)
with A_k = s_k*ln(relu(pd)+eps), C_k = s_k*ln(dm+eps). Using angle addition the
(N, HW, 256) intermediate never exists:
  out[n,hw,h] = sum_k U[k,(n,h)]*cosC[k,hw] + V[k,(n,h)]*sinC[k,hw]
  U = sinA*We + cosA*Wo,  V = sinA*Wo - cosA*We   (We = W[0::2], Wo = W[1::2])
which is one (256 x M) @ (256 x HW) matmul per core.

Trig arguments reach +-1600 rad; ACT Sin is only valid on [-pi, pi], so angles
are computed in "turns" (tau = angle/2pi) via a K=6 bf16-split outer-product
matmul (exact to ~2^-24), range-reduced with f = tau - rint(tau) (the DVE
fp32->int32 copy rounds to nearest on HW), and evaluated as
  sin(2pi tau) = Sin(2pi*f),  cos(2pi tau) = Sin(pi/2 - 2pi*|f|).

Sharding: SN x SH = 4 x 2 cores over (N, HW). Each core computes a full
[M=n_pad*8, hw_per] output block; host reassembles.
"""

import numpy as np

import sys

for p in ("/opt/trn_rl_repo", "/root/.axon_site/_ro/trn_rl_repo"):
    if p not in sys.path:
        sys.path.insert(0, p)

import ml_dtypes
from contextlib import ExitStack

from concourse import bacc, mybir, tile
from concourse.bass_utils import run_bass_kernel_spmd

F32 = mybir.dt.float32
F32R = mybir.dt.float32r
BF16 = mybir.dt.bfloat16
I32 = mybir.dt.int32
A = mybir.AluOpType
AF = mybir.ActivationFunctionType

# ---- problem constants (hardcoded; kernel.py must be self-contained) ----
N_TOT, H_DM, W_DM = 300, 24, 80
HW_TOT = H_DM * W_DM  # 1920
HEADS = 8
ED = 256  # embed dim
K = ED // 2  # 128 frequencies
EPS = 1e-5
SCALE = 100.0
TEMPERATURE = 10000.0
TWO_PI = 2.0 * np.pi

# ---- sharding config ----
SN, SH = 4, 2  # cores = SN * SH = 8
n_per = -(-N_TOT // SN)  # 75
n_pad = n_per + (n_per % 2)  # 76 (even, for clean [dm_rows, pd_cols] packing)
hw_per = HW_TOT // SH  # 960
dm_rows = 128 // SH  # 64
pd_cols = -(-n_pad // dm_rows)  # 2
M = n_pad * HEADS  # 608
CH = 480  # hw chunk width (psum-bank sized)
n_chunks = hw_per // CH
LPW = 15 + pd_cols  # logpack width

_m_tiles = []
_ms = 0
while _ms < M:
    _m_tiles.append((_ms, min(128, M - _ms)))
    _ms += 128


def _sigma_splits():
    k = np.arange(K)
    dim_t = (TEMPERATURE ** (k.astype(np.float32) * 2.0 / ED)).astype(np.float32)
    sigma = (SCALE / dim_t.astype(np.float64)) / TWO_PI
    s0 = sigma.astype(ml_dtypes.bfloat16)
    r1 = sigma - s0.astype(np.float64)
    s1 = r1.astype(ml_dtypes.bfloat16)
    r2 = r1 - s1.astype(np.float64)
    s2 = r2.astype(ml_dtypes.bfloat16)
    # rhs rows are [c0,c0,c0,c1,c1,c2]; pair with sigma rows below
    return np.stack([s0, s1, s2, s0, s1, s0]).astype(ml_dtypes.bfloat16)  # [6,128]


def _build_program():
    nc = bacc.Bacc("TRN2", target_bir_lowering=False, debug=False)

    lp_d = nc.dram_tensor("logpack", [dm_rows, LPW], F32, kind="ExternalInput")
    we_d = nc.dram_tensor("we", [K, HEADS], F32, kind="ExternalInput")
    wo_d = nc.dram_tensor("wo", [K, HEADS], F32, kind="ExternalInput")
    bias_d = nc.dram_tensor("bias_rep", [128, 1], F32, kind="ExternalInput")
    out_d = nc.dram_tensor("out", [M, hw_per], F32, kind="ExternalOutput")
    sig_d = nc.inline_tensor(np.ascontiguousarray(_sigma_splits()), name="sigma6")

    with tile.TileContext(nc) as tc, ExitStack() as ctx:
        sb = ctx.enter_context(tc.tile_pool(name="sb", bufs=1))
        sb2 = ctx.enter_context(tc.tile_pool(name="sb2", bufs=2))
        ps = ctx.enter_context(tc.tile_pool(name="ps", bufs=1, space="PSUM"))
        ps2 = ctx.enter_context(tc.tile_pool(name="ps2", bufs=2, space="PSUM"))
        pso = ctx.enter_context(tc.tile_pool(name="pso", bufs=4, space="PSUM"))

        # ---- constants ----
        def const_tile(val, tag):
            t = sb.tile((128, 1), F32, tag=tag)
            nc.vector.memset(t[:], val)
            return t

        twopi_c = const_tile(TWO_PI, "c_2pi")
        negtwopi_c = const_tile(-TWO_PI, "c_n2pi")
        halfpi_c = const_tile(np.pi / 2, "c_hpi")

        # input DMAs spread across engine queues; lp first (critical path)
        lp = sb.tile((dm_rows, LPW), F32, tag="lp")
        nc.sync.dma_start(lp[:], lp_d[:])
        lhs_s = sb.tile((6, K), BF16, tag="lhs_s")
        nc.scalar.dma_start(lhs_s[:], sig_d[:])
        we_t = sb.tile((K, HEADS), F32, tag="we")
        nc.gpsimd.dma_start(we_t[:], we_d[:])
        wo_t = sb.tile((K, HEADS), F32, tag="wo")
        nc.scalar.dma_start(wo_t[:], wo_d[:])
        bias_t = sb.tile((128, 1), F32, tag="bias")
        nc.gpsimd.dma_start(bias_t[:], bias_d[:])

        # ---- logs: lp = [dm | pd] packed; relu(pd), +eps, ln ----
        nc.vector.tensor_scalar(lp[:, 15:], lp[:, 15:], 0.0, None, A.max)
        nc.vector.tensor_scalar(lp[:], lp[:], EPS, None, A.add)
        lnv = sb.tile((dm_rows, LPW), F32, tag="lnv")
        nc.scalar.activation(lnv[:], lp[:], AF.Ln)

        # ---- 3-way bf16 split of the logs ----
        b0 = sb.tile((dm_rows, LPW), BF16, tag="b0")
        nc.vector.tensor_copy(b0[:], lnv[:])
        r1 = sb.tile((dm_rows, LPW), F32, tag="r1")
        nc.vector.tensor_tensor(r1[:], lnv[:], b0[:], A.subtract)
        b1 = sb.tile((dm_rows, LPW), BF16, tag="b1")
        nc.vector.tensor_copy(b1[:], r1[:])
        r2 = sb.tile((dm_rows, LPW), F32, tag="r2")
        nc.vector.tensor_tensor(r2[:], r1[:], b1[:], A.subtract)
        b2 = sb.tile((dm_rows, LPW), BF16, tag="b2")
        nc.vector.tensor_copy(b2[:], r2[:])

        # ---- assemble split-product rhs rows [c0,c0,c0,c1,c1,c2] ----
        rhs_c = sb.tile((6, hw_per), BF16, tag="rhs_c")
        rhs_a = sb.tile((6, n_pad), BF16, tag="rhs_a")
        p_a = n_pad // pd_cols
        _qeng = [nc.sync, nc.scalar, nc.gpsimd]
        _qi = 0
        for bt, lo, hi in ((b0, 0, 3), (b1, 3, 5), (b2, 5, 6)):
            for r in range(lo, hi):
                _qeng[_qi % 3].dma_start(
                    rhs_c[r : r + 1, :].rearrange("r (p j) -> r p j", j=15),
                    bt[:, 0:15],
                )
                _qeng[(_qi + 1) % 3].dma_start(
                    rhs_a[r : r + 1, :].rearrange("r (p j) -> r p j", j=pd_cols),
                    bt[0:p_a, 15:LPW],
                )
                _qi += 2

        # ---- helper: tau psum -> (sin, cos) via range reduction ----
        def reduce_and_trig(ps_t, width, sin_ap, cos_ap, tag):
            q = sb2.tile((K, width), I32, tag=f"q{tag}")
            nc.vector.tensor_copy(q[:], ps_t[:])  # rint on HW
            f = sb2.tile((K, width), F32, tag=f"f{tag}")
            nc.vector.tensor_tensor(f[:], ps_t[:], q[:], A.subtract)
            u = sb2.tile((K, width), F32, tag=f"u{tag}")
            nc.vector.tensor_scalar(
                u[:].bitcast(I32), f[:].bitcast(I32), 0x7FFFFFFF, None, A.bitwise_and
            )
            nc.scalar.activation(sin_ap, f[:], AF.Sin, scale=twopi_c[:])
            nc.scalar.activation(
                cos_ap, u[:], AF.Sin, bias=halfpi_c[:], scale=negtwopi_c[:]
            )

        # ---- A-side grid ----
        ps_a = ps.tile((K, n_pad), F32, tag="psa")
        nc.tensor.matmul(ps_a[:], lhs_s[:], rhs_a[:], start=True, stop=True)
        sinA = sb.tile((K, n_pad), F32, tag="sinA")
        cosA = sb.tile((K, n_pad), F32, tag="cosA")
        reduce_and_trig(ps_a, n_pad, sinA[:], cosA[:], "a")

        # ---- T build: U = sinA*We + cosA*Wo, V = sinA*Wo - cosA*We ----
        U = sb.tile((K, M), F32R, tag="U")
        V = sb.tile((K, M), F32R, tag="V")
        tmp1 = sb.tile((K, M), F32, tag="tmp1")
        tmp2 = sb.tile((K, M), F32, tag="tmp2")

        def bcast_n(t):  # [K, n_pad] -> [K, n_pad, HEADS]
            return t[:].unsqueeze(2).to_broadcast((K, n_pad, HEADS))

        def bcast_h(t):  # [K, HEADS] -> [K, n_pad, HEADS]
            return t[:].unsqueeze(1).to_broadcast((K, n_pad, HEADS))

        def r3(t):  # [K, M] viewed as [K, n_pad, HEADS]
            return t[:].rearrange("p (n h) -> p n h", h=HEADS)

        # U on VectorE, V on GpSimd (both SBUF-only) to run concurrently
        tmp3 = sb.tile((K, M), F32, tag="tmp3")
        tmp4 = sb.tile((K, M), F32, tag="tmp4")
        nc.vector.tensor_tensor(r3(tmp1), bcast_n(sinA), bcast_h(we_t), A.mult)
        nc.vector.tensor_tensor(r3(tmp2), bcast_n(cosA), bcast_h(wo_t), A.mult)
        nc.vector.tensor_tensor(U[:], tmp1[:], tmp2[:], A.add)
        nc.gpsimd.tensor_tensor(r3(tmp3), bcast_n(sinA), bcast_h(wo_t), A.mult)
        nc.gpsimd.tensor_tensor(r3(tmp4), bcast_n(cosA), bcast_h(we_t), A.mult)
        nc.gpsimd.tensor_tensor(V[:], tmp3[:], tmp4[:], A.subtract)

        # ---- C-side grid (chunked) + main matmul + relu + store ----
        cs_sin = sb.tile((K, hw_per), F32R, tag="cs_sin")
        cs_cos = sb.tile((K, hw_per), F32R, tag="cs_cos")
        for ci in range(n_chunks):
            sl = slice(ci * CH, (ci + 1) * CH)
            ps_c = ps2.tile((K, CH), F32, tag="psc")
            nc.tensor.matmul(ps_c[:], lhs_s[:], rhs_c[:, sl], start=True, stop=True)
            reduce_and_trig(ps_c, CH, cs_sin[:, sl], cs_cos[:, sl], "c")

            for ms, mr in _m_tiles:
                ps_o = pso.tile((128, CH), F32, tag="pso")
                nc.tensor.matmul(
                    ps_o[:mr, :], U[:, ms : ms + mr], cs_cos[:, sl],
                    start=True, stop=False,
                )
                nc.tensor.matmul(
                    ps_o[:mr, :], V[:, ms : ms + mr], cs_sin[:, sl],
                    start=False, stop=True,
                )
                ob = sb2.tile((128, CH), F32, tag="ob")
                ti = ci * len(_m_tiles) + (ms // 128)
                if ti % 2 == 0:
                    nc.scalar.activation(
                        ob[:mr, :], ps_o[:mr, :], AF.Relu, bias=bias_t[0:mr]
                    )
                else:
                    nc.vector.tensor_scalar(
                        ob[:mr, :], ps_o[:mr, :], bias_t[0:mr], 0.0, A.add, A.max
                    )
                _qeng[ti % 3].dma_start(out_d[ms : ms + mr, sl], ob[:mr, :])

    nc.finalize()
    return nc


_NC = None


def _get_nc():
    global _NC
    if _NC is None:
        _NC = _build_program()
    return _NC


def _make_in_maps(predict_depth, depth_map, W, b):
    pd = np.asarray(predict_depth, np.float32).reshape(N_TOT)
    dm = np.asarray(depth_map, np.float32).reshape(128, 15)
    W = np.asarray(W, np.float32)
    b = np.asarray(b, np.float32)
    we = np.ascontiguousarray(W[0::2, :])
    wo = np.ascontiguousarray(W[1::2, :])
    bias_rep = np.ascontiguousarray(np.tile(b, 16)[:, None])

    in_maps = []
    for c in range(SN * SH):
        ni, hi = c // SH, c % SH
        pd_sl = pd[ni * n_per : ni * n_per + n_per]
        pd_pack = np.zeros(dm_rows * pd_cols, np.float32)
        pd_pack[: pd_sl.size] = pd_sl
        logpack = np.concatenate(
            [dm[hi * dm_rows : (hi + 1) * dm_rows], pd_pack.reshape(dm_rows, pd_cols)],
            axis=1,
        )
        in_maps.append(
            {
                "logpack": np.ascontiguousarray(logpack),
                "we": we,
                "wo": wo,
                "bias_rep": bias_rep,
            }
        )
    return in_maps


def _run(inputs, trace=False):
    nc = _get_nc()
    in_maps = _make_in_maps(**inputs)
    res = run_bass_kernel_spmd(
        nc, in_maps, core_ids=list(range(SN * SH)), trace=trace
    )
    out = np.empty((HEADS, N_TOT, HW_TOT), np.float32)
    for c in range(SN * SH):
        ni, hi = c // SH, c % SH
        blk = res.results[c]["out"].reshape(n_pad, HEADS, hw_per).transpose(1, 0, 2)
        n0 = ni * n_per
        n_cnt = min(n_per, N_TOT - n0)
        out[:, n0 : n0 + n_cnt, hi * hw_per : (hi + 1) * hw_per] = blk[:, :n_cnt, :]
    return out, res


def kernel(predict_depth, depth_map, W, b):
    out, _ = _run(
        {"predict_depth": predict_depth, "depth_map": depth_map, "W": W, "b": b}
    )
    return out


# revision 11
# speedup vs baseline: 1.2991x; 1.1954x over previous
"""DepthRelationEmbedding Trainium2 kernel.

Math: out[h, n, hw] = relu( sum_d pos[n,hw,d] * W[d,h] + b[h] ) where pos is the
interleaved sin/cos embedding of delta[n,hw] = ln((relu(pd[n])+eps)/(dm[hw]+eps)).

Key identity: the embedding angle separates: angle_k(n,hw) = A_k(n) - C_k(hw)
with A_k = s_k*ln(relu(pd)+eps), C_k = s_k*ln(dm+eps). Using angle addition the
(N, HW, 256) intermediate never exists:
  out[n,hw,h] = sum_k U[k,(n,h)]*cosC[k,hw] + V[k,(n,h)]*sinC[k,hw]
  U = sinA*We + cosA*Wo,  V = sinA*Wo - cosA*We   (We = W[0::2], Wo = W[1::2])
which is one (256 x M) @ (256 x HW) matmul per core.

Trig arguments reach +-1600 rad; ACT Sin is only valid on [-pi, pi], so angles
are computed in "turns" (tau = angle/2pi) via a K=6 bf16-split outer-product
matmul (exact to ~2^-24), range-reduced with f = tau - rint(tau) (the DVE
fp32->int32 copy rounds to nearest on HW), and evaluated as
  sin(2pi tau) = Sin(2pi*f),  cos(2pi tau) = Sin(pi/2 - 2pi*|f|).

Sharding: SN x SH = 4 x 2 cores over (N, HW). Each core computes a full
[M=n_pad*8, hw_per] output block; host reassembles.
"""

import numpy as np

import sys

for p in ("/opt/trn_rl_repo", "/root/.axon_site/_ro/trn_rl_repo"):
    if p not in sys.path:
        sys.path.insert(0, p)

import ml_dtypes
from contextlib import ExitStack

from concourse import bacc, mybir, tile
from concourse.bass_utils import run_bass_kernel_spmd

F32 = mybir.dt.float32
F32R = mybir.dt.float32r
BF16 = mybir.dt.bfloat16
I32 = mybir.dt.int32
A = mybir.AluOpType
AF = mybir.ActivationFunctionType

# ---- problem constants (hardcoded; kernel.py must be self-contained) ----
N_TOT, H_DM, W_DM = 300, 24, 80
HW_TOT = H_DM * W_DM  # 1920
HEADS = 8
ED = 256  # embed dim
K = ED // 2  # 128 frequencies
EPS = 1e-5
SCALE = 100.0
TEMPERATURE = 10000.0
TWO_PI = 2.0 * np.pi

# ---- sharding config ----
SN, SH = 4, 2  # cores = SN * SH = 8
n_per = -(-N_TOT // SN)  # 75
n_pad = n_per + (n_per % 2)  # 76 (even, for clean [dm_rows, pd_cols] packing)
hw_per = HW_TOT // SH  # 960
dm_rows = 128 // SH  # 64
pd_cols = -(-n_pad // dm_rows)  # 2
M = n_pad * HEADS  # 608
CH = 480  # hw chunk width (psum-bank sized)
n_chunks = hw_per // CH
LPW = 15 + pd_cols  # logpack width

_m_tiles = []
_ms = 0
while _ms < M:
    _m_tiles.append((_ms, min(128, M - _ms)))
    _ms += 128


def _sigma_row():
    k = np.arange(K)
    dim_t = (TEMPERATURE ** (k.astype(np.float32) * 2.0 / ED)).astype(np.float32)
    sigma = (SCALE / dim_t.astype(np.float64)) / TWO_PI
    return sigma.astype(np.float32)[None, :]  # [1,128]


def _build_program():
    nc = bacc.Bacc("TRN2", target_bir_lowering=False, debug=False)

    lp_d = nc.dram_tensor("logpack", [dm_rows, LPW], F32, kind="ExternalInput")
    we_d = nc.dram_tensor("we", [K, HEADS], F32, kind="ExternalInput")
    wo_d = nc.dram_tensor("wo", [K, HEADS], F32, kind="ExternalInput")
    bias_d = nc.dram_tensor("bias_rep", [128, 1], F32, kind="ExternalInput")
    out_d = nc.dram_tensor("out", [M, hw_per], F32, kind="ExternalOutput")
    sig_d = nc.inline_tensor(np.ascontiguousarray(_sigma_row()), name="sigma1")

    with tile.TileContext(nc) as tc, ExitStack() as ctx:
        sb = ctx.enter_context(tc.tile_pool(name="sb", bufs=1))
        sb2 = ctx.enter_context(tc.tile_pool(name="sb2", bufs=2))
        ps = ctx.enter_context(tc.tile_pool(name="ps", bufs=1, space="PSUM"))
        ps2 = ctx.enter_context(tc.tile_pool(name="ps2", bufs=2, space="PSUM"))
        pso = ctx.enter_context(tc.tile_pool(name="pso", bufs=4, space="PSUM"))

        # ---- constants ----
        def const_tile(val, tag):
            t = sb.tile((128, 1), F32, tag=tag)
            nc.vector.memset(t[:], val)
            return t

        twopi_c = const_tile(TWO_PI, "c_2pi")
        negtwopi_c = const_tile(-TWO_PI, "c_n2pi")
        halfpi_c = const_tile(np.pi / 2, "c_hpi")

        # input DMAs spread across engine queues; lp first (critical path)
        lp = sb.tile((dm_rows, LPW), F32, tag="lp")
        nc.sync.dma_start(lp[:], lp_d[:])
        lhs_s = sb.tile((1, K), F32, tag="lhs_s")
        nc.scalar.dma_start(lhs_s[:], sig_d[:])
        we_t = sb.tile((K, HEADS), F32, tag="we")
        nc.gpsimd.dma_start(we_t[:], we_d[:])
        wo_t = sb.tile((K, HEADS), F32, tag="wo")
        nc.scalar.dma_start(wo_t[:], wo_d[:])
        bias_t = sb.tile((128, 1), F32, tag="bias")
        nc.gpsimd.dma_start(bias_t[:], bias_d[:])

        # ---- logs: lp = [dm | pd] packed; relu(pd), +eps, ln ----
        nc.vector.tensor_scalar(lp[:, 15:], lp[:, 15:], 0.0, None, A.max)
        nc.vector.tensor_scalar(lp[:], lp[:], EPS, None, A.add)
        lnv = sb.tile((dm_rows, LPW), F32, tag="lnv")
        nc.scalar.activation(lnv[:], lp[:], AF.Ln)

        # ---- flatten logs to single-partition rows for K=1 outer products ----
        rhs_c = sb.tile((1, hw_per), F32, tag="rhs_c")
        rhs_a = sb.tile((1, n_pad), F32, tag="rhs_a")
        p_a = n_pad // pd_cols
        _qeng = [nc.sync, nc.scalar, nc.gpsimd]
        nc.sync.dma_start(
            rhs_c[0:1, :].rearrange("r (p j) -> r p j", j=15), lnv[:, 0:15]
        )
        nc.scalar.dma_start(
            rhs_a[0:1, :].rearrange("r (p j) -> r p j", j=pd_cols),
            lnv[0:p_a, 15:LPW],
        )

        # ---- helper: tau psum -> (sin, cos) via range reduction ----
        def reduce_and_trig(ps_t, width, sin_ap, cos_ap, tag):
            q = sb2.tile((K, width), I32, tag=f"q{tag}")
            nc.vector.tensor_copy(q[:], ps_t[:])  # rint on HW
            f = sb2.tile((K, width), F32, tag=f"f{tag}")
            nc.vector.tensor_tensor(f[:], ps_t[:], q[:], A.subtract)
            u = sb2.tile((K, width), F32, tag=f"u{tag}")
            nc.vector.tensor_scalar(
                u[:].bitcast(I32), f[:].bitcast(I32), 0x7FFFFFFF, None, A.bitwise_and
            )
            nc.scalar.activation(sin_ap, f[:], AF.Sin, scale=twopi_c[:])
            nc.scalar.activation(
                cos_ap, u[:], AF.Sin, bias=halfpi_c[:], scale=negtwopi_c[:]
            )

        # ---- A-side grid ----
        ps_a = ps.tile((K, n_pad), F32, tag="psa")
        nc.tensor.matmul(ps_a[:], lhs_s[:], rhs_a[:], start=True, stop=True)
        sinA = sb.tile((K, n_pad), F32, tag="sinA")
        cosA = sb.tile((K, n_pad), F32, tag="cosA")
        reduce_and_trig(ps_a, n_pad, sinA[:], cosA[:], "a")

        # ---- T build: U = sinA*We + cosA*Wo, V = sinA*Wo - cosA*We ----
        U = sb.tile((K, M), F32R, tag="U")
        V = sb.tile((K, M), F32R, tag="V")
        tmp1 = sb.tile((K, M), F32, tag="tmp1")
        tmp2 = sb.tile((K, M), F32, tag="tmp2")

        def bcast_n(t):  # [K, n_pad] -> [K, n_pad, HEADS]
            return t[:].unsqueeze(2).to_broadcast((K, n_pad, HEADS))

        def bcast_h(t):  # [K, HEADS] -> [K, n_pad, HEADS]
            return t[:].unsqueeze(1).to_broadcast((K, n_pad, HEADS))

        def r3(t):  # [K, M] viewed as [K, n_pad, HEADS]
            return t[:].rearrange("p (n h) -> p n h", h=HEADS)

        def build_T():
            nc.vector.tensor_tensor(r3(tmp1), bcast_n(sinA), bcast_h(we_t), A.mult)
            nc.vector.tensor_tensor(r3(tmp2), bcast_n(cosA), bcast_h(wo_t), A.mult)
            nc.vector.tensor_tensor(U[:], tmp1[:], tmp2[:], A.add)
            nc.vector.tensor_tensor(r3(tmp1), bcast_n(sinA), bcast_h(wo_t), A.mult)
            nc.vector.tensor_tensor(r3(tmp2), bcast_n(cosA), bcast_h(we_t), A.mult)
            nc.vector.tensor_tensor(V[:], tmp1[:], tmp2[:], A.subtract)

        # ---- C-side grids: tau matmuls early (PE idle), reduction chunk0
        #      before T-build so ACT trig overlaps DVE T-build ----
        cs_sin = sb.tile((K, hw_per), F32R, tag="cs_sin")
        cs_cos = sb.tile((K, hw_per), F32R, tag="cs_cos")
        ps_cs = []
        for ci in range(n_chunks):
            sl = slice(ci * CH, (ci + 1) * CH)
            ps_c = ps2.tile((K, CH), F32, tag="psc")
            nc.tensor.matmul(ps_c[:], lhs_s[:], rhs_c[:, sl], start=True, stop=True)
            ps_cs.append(ps_c)
        reduce_and_trig(ps_cs[0], CH, cs_sin[:, 0:CH], cs_cos[:, 0:CH], "c")

        build_T()

        for ci in range(1, n_chunks):
            sl = slice(ci * CH, (ci + 1) * CH)
            reduce_and_trig(ps_cs[ci], CH, cs_sin[:, sl], cs_cos[:, sl], "c")

        # ---- main matmuls + relu + store (fused per-m output DMA) ----
        obs = {}
        for ci in range(n_chunks):
            sl = slice(ci * CH, (ci + 1) * CH)
            for mi, (ms, mr) in enumerate(_m_tiles):
                ps_o = pso.tile((128, CH), F32, tag="pso")
                nc.tensor.matmul(
                    ps_o[:mr, :], U[:, ms : ms + mr], cs_cos[:, sl],
                    start=True, stop=False,
                )
                nc.tensor.matmul(
                    ps_o[:mr, :], V[:, ms : ms + mr], cs_sin[:, sl],
                    start=False, stop=True,
                )
                if ci == 0:
                    ob_new = sb2.tile((128, hw_per), F32, tag="ob")
                    obs[mi] = ob_new
                ob = obs[mi]
                ti = ci * len(_m_tiles) + mi
                if ti % 2 == 0:
                    nc.scalar.activation(
                        ob[:mr, sl], ps_o[:mr, :], AF.Relu, bias=bias_t[0:mr]
                    )
                else:
                    nc.vector.tensor_scalar(
                        ob[:mr, sl], ps_o[:mr, :], bias_t[0:mr], 0.0, A.add, A.max
                    )
                if ci == n_chunks - 1:
                    _qeng[mi % 3].dma_start(out_d[ms : ms + mr, :], ob[:mr, :])

    nc.finalize()
    return nc


_NC = None


def _get_nc():
    global _NC
    if _NC is None:
        _NC = _build_program()
    return _NC


def _make_in_maps(predict_depth, depth_map, W, b):
    pd = np.asarray(predict_depth, np.float32).reshape(N_TOT)
    dm = np.asarray(depth_map, np.float32).reshape(128, 15)
    W = np.asarray(W, np.float32)
    b = np.asarray(b, np.float32)
    we = np.ascontiguousarray(W[0::2, :])
    wo = np.ascontiguousarray(W[1::2, :])
    bias_rep = np.ascontiguousarray(np.tile(b, 16)[:, None])

    in_maps = []
    for c in range(SN * SH):
        ni, hi = c // SH, c % SH
        pd_sl = pd[ni * n_per : ni * n_per + n_per]
        pd_pack = np.zeros(dm_rows * pd_cols, np.float32)
        pd_pack[: pd_sl.size] = pd_sl
        logpack = np.concatenate(
            [dm[hi * dm_rows : (hi + 1) * dm_rows], pd_pack.reshape(dm_rows, pd_cols)],
            axis=1,
        )
        in_maps.append(
            {
                "logpack": np.ascontiguousarray(logpack),
                "we": we,
                "wo": wo,
                "bias_rep": bias_rep,
            }
        )
    return in_maps


def _run(inputs, trace=False):
    nc = _get_nc()
    in_maps = _make_in_maps(**inputs)
    res = run_bass_kernel_spmd(
        nc, in_maps, core_ids=list(range(SN * SH)), trace=trace
    )
    out = np.empty((HEADS, N_TOT, HW_TOT), np.float32)
    for c in range(SN * SH):
        ni, hi = c // SH, c % SH
        blk = res.results[c]["out"].reshape(n_pad, HEADS, hw_per).transpose(1, 0, 2)
        n0 = ni * n_per
        n_cnt = min(n_per, N_TOT - n0)
        out[:, n0 : n0 + n_cnt, hi * hw_per : (hi + 1) * hw_per] = blk[:, :n_cnt, :]
    return out, res


def kernel(predict_depth, depth_map, W, b):
    out, _ = _run(
        {"predict_depth": predict_depth, "depth_map": depth_map, "W": W, "b": b}
    )
    return out


# revision 12
# speedup vs baseline: 1.3922x; 1.0717x over previous
"""DepthRelationEmbedding Trainium2 kernel.

Math: out[h, n, hw] = relu( sum_d pos[n,hw,d] * W[d,h] + b[h] ) where pos is the
interleaved sin/cos embedding of delta[n,hw] = ln((relu(pd[n])+eps)/(dm[hw]+eps)).

Key identity: the embedding angle separates: angle_k(n,hw) = A_k(n) - C_k(hw)
with A_k = s_k*ln(relu(pd)+eps), C_k = s_k*ln(dm+eps). Using angle addition the
(N, HW, 256) intermediate never exists:
  out[n,hw,h] = sum_k U[k,(n,h)]*cosC[k,hw] + V[k,(n,h)]*sinC[k,hw]
  U = sinA*We + cosA*Wo,  V = sinA*Wo - cosA*We   (We = W[0::2], Wo = W[1::2])
which is one (256 x M) @ (256 x HW) matmul per core.

Trig arguments reach +-1600 rad; ACT Sin is only valid on [-pi, pi], so angles
are computed in "turns" (tau = angle/2pi) via a K=6 bf16-split outer-product
matmul (exact to ~2^-24), range-reduced with f = tau - rint(tau) (the DVE
fp32->int32 copy rounds to nearest on HW), and evaluated as
  sin(2pi tau) = Sin(2pi*f),  cos(2pi tau) = Sin(pi/2 - 2pi*|f|).

Sharding: SN x SH = 4 x 2 cores over (N, HW). Each core computes a full
[M=n_pad*8, hw_per] output block; host reassembles.
"""

import numpy as np

import sys

for p in ("/opt/trn_rl_repo", "/root/.axon_site/_ro/trn_rl_repo"):
    if p not in sys.path:
        sys.path.insert(0, p)

import ml_dtypes
from contextlib import ExitStack

from concourse import bacc, mybir, tile
from concourse.bass_utils import run_bass_kernel_spmd

F32 = mybir.dt.float32
F32R = mybir.dt.float32r
BF16 = mybir.dt.bfloat16
I32 = mybir.dt.int32
A = mybir.AluOpType
AF = mybir.ActivationFunctionType

# ---- problem constants (hardcoded; kernel.py must be self-contained) ----
N_TOT, H_DM, W_DM = 300, 24, 80
HW_TOT = H_DM * W_DM  # 1920
HEADS = 8
ED = 256  # embed dim
K = ED // 2  # 128 frequencies
EPS = 1e-5
SCALE = 100.0
TEMPERATURE = 10000.0
TWO_PI = 2.0 * np.pi

# ---- sharding config ----
SN, SH = 4, 2  # cores = SN * SH = 8
n_per = -(-N_TOT // SN)  # 75
n_pad = n_per + (n_per % 2)  # 76 (even, for clean [dm_rows, pd_cols] packing)
hw_per = HW_TOT // SH  # 960
dm_rows = 128 // SH  # 64
pd_cols = -(-n_pad // dm_rows)  # 2
M = n_pad * HEADS  # 608
CH = 480  # hw chunk width (psum-bank sized)
n_chunks = hw_per // CH
LPW = 15 + pd_cols  # logpack width

_m_tiles = []
_ms = 0
while _ms < M:
    _m_tiles.append((_ms, min(128, M - _ms)))
    _ms += 128


def _sigma_row():
    k = np.arange(K)
    dim_t = (TEMPERATURE ** (k.astype(np.float32) * 2.0 / ED)).astype(np.float32)
    sigma = (SCALE / dim_t.astype(np.float64)) / TWO_PI
    return sigma.astype(np.float32)[None, :]  # [1,128]


def _build_program():
    nc = bacc.Bacc("TRN2", target_bir_lowering=False, debug=False)

    lp_d = nc.dram_tensor("logpack", [dm_rows, LPW], F32, kind="ExternalInput")
    wew_d = nc.dram_tensor("wew", [K, 2 * HEADS], F32, kind="ExternalInput")
    wow_d = nc.dram_tensor("wow", [K, 2 * HEADS], F32, kind="ExternalInput")
    bias_d = nc.dram_tensor("bias_rep", [128, 1], F32, kind="ExternalInput")
    out_d = nc.dram_tensor("out", [M, hw_per], F32, kind="ExternalOutput")
    sig_d = nc.inline_tensor(np.ascontiguousarray(_sigma_row()), name="sigma1")

    with tile.TileContext(nc) as tc, ExitStack() as ctx:
        sb = ctx.enter_context(tc.tile_pool(name="sb", bufs=1))
        sb2 = ctx.enter_context(tc.tile_pool(name="sb2", bufs=2))
        ps = ctx.enter_context(tc.tile_pool(name="ps", bufs=1, space="PSUM"))
        ps2 = ctx.enter_context(tc.tile_pool(name="ps2", bufs=2, space="PSUM"))
        pso = ctx.enter_context(tc.tile_pool(name="pso", bufs=5, space="PSUM"))

        # ---- constants ----
        def const_tile(val, tag):
            t = sb.tile((128, 1), F32, tag=tag)
            nc.vector.memset(t[:], val)
            return t

        twopi_c = const_tile(TWO_PI, "c_2pi")
        negtwopi_c = const_tile(-TWO_PI, "c_n2pi")
        halfpi_c = const_tile(np.pi / 2, "c_hpi")

        # trigger the natural_log ACT table load at kernel start so the real
        # Ln below doesn't stall on it
        lnwarm = sb.tile((128, 1), F32, tag="lnwarm")
        nc.scalar.activation(lnwarm[:], twopi_c[:], AF.Ln)

        # input DMAs spread across engine queues; lp first (critical path)
        lp = sb.tile((dm_rows, LPW), F32, tag="lp")
        nc.sync.dma_start(lp[:], lp_d[:])
        lhs_s = sb.tile((1, K), F32, tag="lhs_s")
        nc.scalar.dma_start(lhs_s[:], sig_d[:])
        wew_t = sb.tile((K, 2 * HEADS), F32, tag="wew")
        nc.gpsimd.dma_start(wew_t[:], wew_d[:])
        wow_t = sb.tile((K, 2 * HEADS), F32, tag="wow")
        nc.scalar.dma_start(wow_t[:], wow_d[:])
        bias_t = sb.tile((128, 1), F32, tag="bias")
        nc.gpsimd.dma_start(bias_t[:], bias_d[:])

        # ---- logs: lp = [dm | pd] packed; relu(pd), +eps, ln ----
        nc.vector.tensor_scalar(lp[:, 15:], lp[:, 15:], 0.0, None, A.max)
        nc.vector.tensor_scalar(lp[:], lp[:], EPS, None, A.add)
        lnv = sb.tile((dm_rows, LPW), F32, tag="lnv")
        nc.scalar.activation(lnv[:], lp[:], AF.Ln)

        # ---- flatten logs to single-partition rows for K=1 outer products ----
        rhs_c = sb.tile((1, hw_per), F32, tag="rhs_c")
        rhs_a = sb.tile((1, n_pad), F32, tag="rhs_a")
        p_a = n_pad // pd_cols
        _qeng = [nc.sync, nc.scalar, nc.gpsimd]
        nc.sync.dma_start(
            rhs_c[0:1, :].rearrange("r (p j) -> r p j", j=15), lnv[:, 0:15]
        )
        nc.scalar.dma_start(
            rhs_a[0:1, :].rearrange("r (p j) -> r p j", j=pd_cols),
            lnv[0:p_a, 15:LPW],
        )

        # ---- helper: tau psum -> (sin, cos) via range reduction ----
        def reduce_and_trig(ps_t, width, sin_ap, cos_ap, tag):
            q = sb2.tile((K, width), I32, tag=f"q{tag}")
            nc.vector.tensor_copy(q[:], ps_t[:])  # rint on HW
            f = sb2.tile((K, width), F32, tag=f"f{tag}")
            nc.vector.tensor_tensor(f[:], ps_t[:], q[:], A.subtract)
            u = sb2.tile((K, width), F32, tag=f"u{tag}")
            nc.vector.tensor_scalar(
                u[:].bitcast(I32), f[:].bitcast(I32), 0x7FFFFFFF, None, A.bitwise_and
            )
            nc.scalar.activation(sin_ap, f[:], AF.Sin, scale=twopi_c[:])
            nc.scalar.activation(
                cos_ap, u[:], AF.Sin, bias=halfpi_c[:], scale=negtwopi_c[:]
            )

        # ---- A-side grid (sin and cos packed in one tile for 4-op T build) ----
        ps_a = ps.tile((K, n_pad), F32, tag="psa")
        nc.tensor.matmul(ps_a[:], lhs_s[:], rhs_a[:], start=True, stop=True)
        trigA = sb.tile((K, 2 * n_pad), F32, tag="trigA")
        reduce_and_trig(ps_a, n_pad, trigA[:, 0:n_pad], trigA[:, n_pad:], "a")

        # ---- T build: U = sinA*We + cosA*Wo, V = sinA*Wo - cosA*We ----
        # products as one [K, 2, n_pad, HEADS] broadcast multiply per (U, V)
        U = sb.tile((K, M), F32R, tag="U")
        V = sb.tile((K, M), F32R, tag="V")
        tmp1 = sb.tile((K, 2 * M), F32, tag="tmp1")
        tmp2 = sb.tile((K, 2 * M), F32, tag="tmp2")

        def trig_bc():  # [K, 2*n_pad] -> [K, 2, n_pad, HEADS]
            return (
                trigA[:]
                .rearrange("p (s n) -> p s n", s=2)
                .unsqueeze(3)
                .to_broadcast((K, 2, n_pad, HEADS))
            )

        def w_bc(t):  # [K, 2*HEADS] -> [K, 2, n_pad, HEADS]
            return (
                t[:]
                .rearrange("p (s h) -> p s h", s=2)
                .unsqueeze(2)
                .to_broadcast((K, 2, n_pad, HEADS))
            )

        def r4(t):  # [K, 2*M] viewed as [K, 2, n_pad, HEADS]
            return t[:].rearrange("p (s n h) -> p s n h", s=2, h=HEADS)

        def build_T():
            nc.vector.tensor_tensor(r4(tmp1), trig_bc(), w_bc(wew_t), A.mult)
            nc.vector.tensor_tensor(U[:], tmp1[:, 0:M], tmp1[:, M:], A.add)
            nc.vector.tensor_tensor(r4(tmp2), trig_bc(), w_bc(wow_t), A.mult)
            nc.vector.tensor_tensor(V[:], tmp2[:, 0:M], tmp2[:, M:], A.subtract)

        # ---- C-side grids: tau matmuls early (PE idle), reduction chunk0
        #      before T-build so ACT trig overlaps DVE T-build ----
        cs_sin = sb.tile((K, hw_per), F32R, tag="cs_sin")
        cs_cos = sb.tile((K, hw_per), F32R, tag="cs_cos")
        ps_cs = []
        for ci in range(n_chunks):
            sl = slice(ci * CH, (ci + 1) * CH)
            ps_c = ps2.tile((K, CH), F32, tag="psc")
            nc.tensor.matmul(ps_c[:], lhs_s[:], rhs_c[:, sl], start=True, stop=True)
            ps_cs.append(ps_c)
        reduce_and_trig(ps_cs[0], CH, cs_sin[:, 0:CH], cs_cos[:, 0:CH], "c")

        build_T()

        for ci in range(1, n_chunks):
            sl = slice(ci * CH, (ci + 1) * CH)
            reduce_and_trig(ps_cs[ci], CH, cs_sin[:, sl], cs_cos[:, sl], "c")

        # ---- main matmuls + relu + store (fused per-m output DMA) ----
        obs = {}
        for ci in range(n_chunks):
            sl = slice(ci * CH, (ci + 1) * CH)
            for mi, (ms, mr) in enumerate(_m_tiles):
                ps_o = pso.tile((128, CH), F32, tag="pso")
                nc.tensor.matmul(
                    ps_o[:mr, :], U[:, ms : ms + mr], cs_cos[:, sl],
                    start=True, stop=False,
                )
                nc.tensor.matmul(
                    ps_o[:mr, :], V[:, ms : ms + mr], cs_sin[:, sl],
                    start=False, stop=True,
                )
                if ci == 0:
                    ob_new = sb.tile((128, hw_per), F32, tag=f"ob{mi}")
                    obs[mi] = ob_new
                ob = obs[mi]
                ti = ci * len(_m_tiles) + mi
                if ti % 2 == 0:
                    nc.scalar.activation(
                        ob[:mr, sl], ps_o[:mr, :], AF.Relu, bias=bias_t[0:mr]
                    )
                else:
                    nc.vector.tensor_scalar(
                        ob[:mr, sl], ps_o[:mr, :], bias_t[0:mr], 0.0, A.add, A.max
                    )
                if ci == n_chunks - 1:
                    _qeng[mi % 3].dma_start(out_d[ms : ms + mr, :], ob[:mr, :])

    nc.finalize()
    return nc


_NC = None


def _get_nc():
    global _NC
    if _NC is None:
        _NC = _build_program()
    return _NC


def _make_in_maps(predict_depth, depth_map, W, b):
    pd = np.asarray(predict_depth, np.float32).reshape(N_TOT)
    dm = np.asarray(depth_map, np.float32).reshape(128, 15)
    W = np.asarray(W, np.float32)
    b = np.asarray(b, np.float32)
    we = W[0::2, :]
    wo = W[1::2, :]
    wew = np.ascontiguousarray(np.stack([we, wo], axis=1).reshape(K, 2 * HEADS))
    wow = np.ascontiguousarray(np.stack([wo, we], axis=1).reshape(K, 2 * HEADS))
    bias_rep = np.ascontiguousarray(np.tile(b, 16)[:, None])

    in_maps = []
    for c in range(SN * SH):
        ni, hi = c // SH, c % SH
        pd_sl = pd[ni * n_per : ni * n_per + n_per]
        pd_pack = np.zeros(dm_rows * pd_cols, np.float32)
        pd_pack[: pd_sl.size] = pd_sl
        logpack = np.concatenate(
            [dm[hi * dm_rows : (hi + 1) * dm_rows], pd_pack.reshape(dm_rows, pd_cols)],
            axis=1,
        )
        in_maps.append(
            {
                "logpack": np.ascontiguousarray(logpack),
                "wew": wew,
                "wow": wow,
                "bias_rep": bias_rep,
            }
        )
    return in_maps


def _run(inputs, trace=False):
    nc = _get_nc()
    in_maps = _make_in_maps(**inputs)
    res = run_bass_kernel_spmd(
        nc, in_maps, core_ids=list(range(SN * SH)), trace=trace
    )
    out = np.empty((HEADS, N_TOT, HW_TOT), np.float32)
    for c in range(SN * SH):
        ni, hi = c // SH, c % SH
        blk = res.results[c]["out"].reshape(n_pad, HEADS, hw_per).transpose(1, 0, 2)
        n0 = ni * n_per
        n_cnt = min(n_per, N_TOT - n0)
        out[:, n0 : n0 + n_cnt, hi * hw_per : (hi + 1) * hw_per] = blk[:, :n_cnt, :]
    return out, res


def kernel(predict_depth, depth_map, W, b):
    out, _ = _run(
        {"predict_depth": predict_depth, "depth_map": depth_map, "W": W, "b": b}
    )
    return out


# revision 13
# speedup vs baseline: 1.5491x; 1.1127x over previous
"""DepthRelationEmbedding Trainium2 kernel.

Math: out[h, n, hw] = relu( sum_d pos[n,hw,d] * W[d,h] + b[h] ) where pos is the
interleaved sin/cos embedding of delta[n,hw] = ln((relu(pd[n])+eps)/(dm[hw]+eps)).

Key identity: the embedding angle separates: angle_k(n,hw) = A_k(n) - C_k(hw)
with A_k = s_k*ln(relu(pd)+eps), C_k = s_k*ln(dm+eps). Using angle addition the
(N, HW, 256) intermediate never exists:
  out[n,hw,h] = sum_k U[k,(n,h)]*cosC[k,hw] + V[k,(n,h)]*sinC[k,hw]
  U = sinA*We + cosA*Wo,  V = sinA*Wo - cosA*We   (We = W[0::2], Wo = W[1::2])
which is one (256 x M) @ (256 x HW) matmul per core.

Trig arguments reach +-1600 rad; ACT Sin is only valid on [-pi, pi], so angles
are computed in "turns" (tau = angle/2pi) via a K=6 bf16-split outer-product
matmul (exact to ~2^-24), range-reduced with f = tau - rint(tau) (the DVE
fp32->int32 copy rounds to nearest on HW), and evaluated as
  sin(2pi tau) = Sin(2pi*f),  cos(2pi tau) = Sin(pi/2 - 2pi*|f|).

Sharding: SN x SH = 4 x 2 cores over (N, HW). Each core computes a full
[M=n_pad*8, hw_per] output block; host reassembles.
"""

import numpy as np

import sys

for p in ("/opt/trn_rl_repo", "/root/.axon_site/_ro/trn_rl_repo"):
    if p not in sys.path:
        sys.path.insert(0, p)

import ml_dtypes
from contextlib import ExitStack

from concourse import bacc, mybir, tile
from concourse.bass_utils import run_bass_kernel_spmd

F32 = mybir.dt.float32
F32R = mybir.dt.float32r
BF16 = mybir.dt.bfloat16
I32 = mybir.dt.int32
A = mybir.AluOpType
AF = mybir.ActivationFunctionType

# ---- problem constants (hardcoded; kernel.py must be self-contained) ----
N_TOT, H_DM, W_DM = 300, 24, 80
HW_TOT = H_DM * W_DM  # 1920
HEADS = 8
ED = 256  # embed dim
K = ED // 2  # 128 frequencies
EPS = 1e-5
SCALE = 100.0
TEMPERATURE = 10000.0
TWO_PI = 2.0 * np.pi

# ---- sharding config ----
SN, SH = 4, 2  # cores = SN * SH = 8
n_per = -(-N_TOT // SN)  # 75
n_pad = n_per + (n_per % 2)  # 76 (even, for clean [dm_rows, pd_cols] packing)
hw_per = HW_TOT // SH  # 960
dm_rows = 128 // SH  # 64
pd_cols = -(-n_pad // dm_rows)  # 2
M = n_pad * HEADS  # 608
CH = 480  # hw chunk width (psum-bank sized)
n_chunks = hw_per // CH
LPW = 15 + pd_cols  # logpack width

_m_tiles = []
_ms = 0
while _ms < M:
    _m_tiles.append((_ms, min(128, M - _ms)))
    _ms += 128


def _sigma_row():
    k = np.arange(K)
    dim_t = (TEMPERATURE ** (k.astype(np.float32) * 2.0 / ED)).astype(np.float32)
    sigma = (SCALE / dim_t.astype(np.float64)) / TWO_PI
    return sigma.astype(np.float32)[None, :]  # [1,128]


def _build_program():
    nc = bacc.Bacc("TRN2", target_bir_lowering=False, debug=False)

    lp_d = nc.dram_tensor("logpack", [dm_rows, LPW], F32, kind="ExternalInput")
    wew_d = nc.dram_tensor("wew", [K, 2 * HEADS], F32, kind="ExternalInput")
    wow_d = nc.dram_tensor("wow", [K, 2 * HEADS], F32, kind="ExternalInput")
    bias_d = nc.dram_tensor("bias_rep", [128, 1], F32, kind="ExternalInput")
    out_d = nc.dram_tensor("out", [M, hw_per], F32, kind="ExternalOutput")
    sig_d = nc.inline_tensor(np.ascontiguousarray(_sigma_row()), name="sigma1")

    with tile.TileContext(nc) as tc, ExitStack() as ctx:
        sb = ctx.enter_context(tc.tile_pool(name="sb", bufs=1))
        sb2 = ctx.enter_context(tc.tile_pool(name="sb2", bufs=2))
        ps = ctx.enter_context(tc.tile_pool(name="ps", bufs=1, space="PSUM"))
        ps2 = ctx.enter_context(tc.tile_pool(name="ps2", bufs=2, space="PSUM"))
        pso = ctx.enter_context(tc.tile_pool(name="pso", bufs=5, space="PSUM"))

        # ---- constants ----
        def const_tile(val, tag):
            t = sb.tile((128, 1), F32, tag=tag)
            nc.vector.memset(t[:], val)
            return t

        twopi_c = const_tile(TWO_PI, "c_2pi")
        negtwopi_c = const_tile(-TWO_PI, "c_n2pi")
        halfpi_c = const_tile(np.pi / 2, "c_hpi")

        # trigger the natural_log ACT table load at kernel start so the real
        # Ln below doesn't stall on it
        lnwarm = sb.tile((128, 1), F32, tag="lnwarm")
        nc.scalar.activation(lnwarm[:], twopi_c[:], AF.Ln)

        # PE warmup: ~5us of dummy matmuls so HAM un-throttles (1.2->2.4 GHz)
        # before the real tau matmuls; they share the psa psum slot so the
        # tau_A matmul simply queues behind them.
        wa = sb.tile((128, 128), BF16, tag="wa")
        wb = sb.tile((128, 512), BF16, tag="wb")
        nc.gpsimd.memset(wa[:], 0)
        nc.gpsimd.memset(wb[:], 0)
        ps_warm = ps.tile((128, 512), F32, tag="psa")
        for _ in range(9):
            nc.tensor.matmul(ps_warm[:], wa[:], wb[:], start=True, stop=True)

        # input DMAs spread across engine queues; lp first (critical path)
        lp = sb.tile((dm_rows, LPW), F32, tag="lp")
        nc.sync.dma_start(lp[:], lp_d[:])
        lhs_s = sb.tile((1, K), F32, tag="lhs_s")
        nc.scalar.dma_start(lhs_s[:], sig_d[:])
        wew_t = sb.tile((K, 2 * HEADS), F32, tag="wew")
        nc.gpsimd.dma_start(wew_t[:], wew_d[:])
        wow_t = sb.tile((K, 2 * HEADS), F32, tag="wow")
        nc.scalar.dma_start(wow_t[:], wow_d[:])
        bias_t = sb.tile((128, 1), F32, tag="bias")
        nc.gpsimd.dma_start(bias_t[:], bias_d[:])

        # ---- logs: lp = [dm | pd] packed; relu(pd), +eps, ln ----
        nc.vector.tensor_scalar(lp[:, 15:], lp[:, 15:], 0.0, None, A.max)
        nc.vector.tensor_scalar(lp[:], lp[:], EPS, None, A.add)
        lnv = sb.tile((dm_rows, LPW), F32, tag="lnv")
        nc.scalar.activation(lnv[:], lp[:], AF.Ln)

        # ---- flatten logs to single-partition rows for K=1 outer products ----
        rhs_c = sb.tile((1, hw_per), F32, tag="rhs_c")
        rhs_a = sb.tile((1, n_pad), F32, tag="rhs_a")
        p_a = n_pad // pd_cols
        _qeng = [nc.sync, nc.scalar, nc.gpsimd]
        nc.sync.dma_start(
            rhs_c[0:1, :].rearrange("r (p j) -> r p j", j=15), lnv[:, 0:15]
        )
        nc.scalar.dma_start(
            rhs_a[0:1, :].rearrange("r (p j) -> r p j", j=pd_cols),
            lnv[0:p_a, 15:LPW],
        )

        # ---- helper: tau psum -> (sin, cos) via range reduction ----
        def reduce_and_trig(ps_t, width, sin_ap, cos_ap, tag):
            q = sb2.tile((K, width), I32, tag=f"q{tag}")
            nc.vector.tensor_copy(q[:], ps_t[:])  # rint on HW
            f = sb2.tile((K, width), F32, tag=f"f{tag}")
            nc.vector.tensor_tensor(f[:], ps_t[:], q[:], A.subtract)
            u = sb2.tile((K, width), F32, tag=f"u{tag}")
            nc.vector.tensor_scalar(
                u[:].bitcast(I32), f[:].bitcast(I32), 0x7FFFFFFF, None, A.bitwise_and
            )
            nc.scalar.activation(sin_ap, f[:], AF.Sin, scale=twopi_c[:])
            nc.scalar.activation(
                cos_ap, u[:], AF.Sin, bias=halfpi_c[:], scale=negtwopi_c[:]
            )

        # ---- A-side grid (sin and cos packed in one tile for 4-op T build) ----
        ps_a = ps.tile((K, n_pad), F32, tag="psa")
        nc.tensor.matmul(ps_a[:], lhs_s[:], rhs_a[:], start=True, stop=True)
        trigA = sb.tile((K, 2 * n_pad), F32, tag="trigA")
        reduce_and_trig(ps_a, n_pad, trigA[:, 0:n_pad], trigA[:, n_pad:], "a")

        # ---- T build: U = sinA*We + cosA*Wo, V = sinA*Wo - cosA*We ----
        # products as one [K, 2, n_pad, HEADS] broadcast multiply per (U, V)
        U = sb.tile((K, M), F32R, tag="U")
        V = sb.tile((K, M), F32R, tag="V")
        tmp1 = sb.tile((K, 2 * M), F32, tag="tmp1")
        tmp2 = sb.tile((K, 2 * M), F32, tag="tmp2")

        def trig_bc():  # [K, 2*n_pad] -> [K, 2, n_pad, HEADS]
            return (
                trigA[:]
                .rearrange("p (s n) -> p s n", s=2)
                .unsqueeze(3)
                .to_broadcast((K, 2, n_pad, HEADS))
            )

        def w_bc(t):  # [K, 2*HEADS] -> [K, 2, n_pad, HEADS]
            return (
                t[:]
                .rearrange("p (s h) -> p s h", s=2)
                .unsqueeze(2)
                .to_broadcast((K, 2, n_pad, HEADS))
            )

        def r4(t):  # [K, 2*M] viewed as [K, 2, n_pad, HEADS]
            return t[:].rearrange("p (s n h) -> p s n h", s=2, h=HEADS)

        def build_U():
            nc.vector.tensor_tensor(r4(tmp1), trig_bc(), w_bc(wew_t), A.mult)
            nc.vector.tensor_tensor(U[:], tmp1[:, 0:M], tmp1[:, M:], A.add)

        def build_V():
            nc.vector.tensor_tensor(r4(tmp2), trig_bc(), w_bc(wow_t), A.mult)
            nc.vector.tensor_tensor(V[:], tmp2[:, 0:M], tmp2[:, M:], A.subtract)

        # ---- C-side grids: tau matmuls early (PE idle), reduction chunk0
        #      before T-build so ACT trig overlaps DVE T-build ----
        cs_sin = sb.tile((K, hw_per), F32R, tag="cs_sin")
        cs_cos = sb.tile((K, hw_per), F32R, tag="cs_cos")
        ps_cs = []
        for ci in range(n_chunks):
            sl = slice(ci * CH, (ci + 1) * CH)
            ps_c = ps2.tile((K, CH), F32, tag="psc")
            nc.tensor.matmul(ps_c[:], lhs_s[:], rhs_c[:, sl], start=True, stop=True)
            ps_cs.append(ps_c)
        reduce_and_trig(ps_cs[0], CH, cs_sin[:, 0:CH], cs_cos[:, 0:CH], "c")
        build_U()
        for ci in range(1, n_chunks):
            sl = slice(ci * CH, (ci + 1) * CH)
            reduce_and_trig(ps_cs[ci], CH, cs_sin[:, sl], cs_cos[:, sl], "c")
        build_V()

        # ---- main matmuls + relu + store (fused per-m output DMA) ----
        obs = {}
        for ci in range(n_chunks):
            sl = slice(ci * CH, (ci + 1) * CH)
            for mi, (ms, mr) in enumerate(_m_tiles):
                ps_o = pso.tile((128, CH), F32, tag="pso")
                nc.tensor.matmul(
                    ps_o[:mr, :], U[:, ms : ms + mr], cs_cos[:, sl],
                    start=True, stop=False,
                )
                nc.tensor.matmul(
                    ps_o[:mr, :], V[:, ms : ms + mr], cs_sin[:, sl],
                    start=False, stop=True,
                )
                if ci == 0:
                    ob_new = sb.tile((128, hw_per), F32, tag=f"ob{mi}")
                    obs[mi] = ob_new
                ob = obs[mi]
                ti = ci * len(_m_tiles) + mi
                if ti % 2 == 0:
                    nc.scalar.activation(
                        ob[:mr, sl], ps_o[:mr, :], AF.Relu, bias=bias_t[0:mr]
                    )
                else:
                    nc.vector.tensor_scalar(
                        ob[:mr, sl], ps_o[:mr, :], bias_t[0:mr], 0.0, A.add, A.max
                    )
                oq = nc.sync if ti % 2 == 0 else nc.scalar
                oq.dma_start(out_d[ms : ms + mr, sl], ob[:mr, sl])

    nc.finalize()
    return nc


_NC = None


def _get_nc():
    global _NC
    if _NC is None:
        _NC = _build_program()
    return _NC


def _make_in_maps(predict_depth, depth_map, W, b):
    pd = np.asarray(predict_depth, np.float32).reshape(N_TOT)
    dm = np.asarray(depth_map, np.float32).reshape(128, 15)
    W = np.asarray(W, np.float32)
    b = np.asarray(b, np.float32)
    we = W[0::2, :]
    wo = W[1::2, :]
    wew = np.ascontiguousarray(np.stack([we, wo], axis=1).reshape(K, 2 * HEADS))
    wow = np.ascontiguousarray(np.stack([wo, we], axis=1).reshape(K, 2 * HEADS))
    bias_rep = np.ascontiguousarray(np.tile(b, 16)[:, None])

    in_maps = []
    for c in range(SN * SH):
        ni, hi = c // SH, c % SH
        pd_sl = pd[ni * n_per : ni * n_per + n_per]
        pd_pack = np.zeros(dm_rows * pd_cols, np.float32)
        pd_pack[: pd_sl.size] = pd_sl
        logpack = np.concatenate(
            [dm[hi * dm_rows : (hi + 1) * dm_rows], pd_pack.reshape(dm_rows, pd_cols)],
            axis=1,
        )
        in_maps.append(
            {
                "logpack": np.ascontiguousarray(logpack),
                "wew": wew,
                "wow": wow,
                "bias_rep": bias_rep,
            }
        )
    return in_maps


def _run(inputs, trace=False):
    nc = _get_nc()
    in_maps = _make_in_maps(**inputs)
    res = run_bass_kernel_spmd(
        nc, in_maps, core_ids=list(range(SN * SH)), trace=trace
    )
    out = np.empty((HEADS, N_TOT, HW_TOT), np.float32)
    for c in range(SN * SH):
        ni, hi = c // SH, c % SH
        blk = res.results[c]["out"].reshape(n_pad, HEADS, hw_per).transpose(1, 0, 2)
        n0 = ni * n_per
        n_cnt = min(n_per, N_TOT - n0)
        out[:, n0 : n0 + n_cnt, hi * hw_per : (hi + 1) * hw_per] = blk[:, :n_cnt, :]
    return out, res


def kernel(predict_depth, depth_map, W, b):
    out, _ = _run(
        {"predict_depth": predict_depth, "depth_map": depth_map, "W": W, "b": b}
    )
    return out
